# revision 26
# baseline (speedup 1.0000x reference)
"""Trainium2 Bass kernel for the conv1d-stack + MLP ragged-sequence model.

Strategy:
  - Pure data parallelism over 8 NeuronCores: 8 samples per core.
  - Samples are sorted by valid length (descending) and dealt round-robin to
    cores, so every core runs an IDENTICAL (SPMD) program whose per-slot
    sizes are the per-rank-group maximum length (exploits raggedness).
  - Convs run on the TensorEngine as float32r matmuls with per-tap PSUM
    accumulation.  All activations are kept PHASE-SPLIT (even/odd time
    samples in separate buffers), which turns every stride-2 conv into
    stride-1 matmul reads — stride-2 moving operands run the PE at half
    rate, stride-1 reach the full warm rate (~0.45 ns/col measured).
  - Layers 5-7 additionally interleave all 8 slots (column = t*8 + s) so
    their matmul reads are single contiguous slabs.
  - Slots are emitted in pairs so PSUM-eviction latency of slot j hides
    behind slot j+1's matmuls.  Bias+ReLU eviction on ScalarE (which also
    does the phase split via strided reads), avgpool on VectorE.
  - The ragged masked-max uses a host-built additive -1e30 mask (a data
    input, SPMD-safe), then the 3-layer MLP runs on-device.
"""

import os
import sys

for _p in ("/opt/trn_rl_repo",):
    if _p not in sys.path and os.path.isdir(_p):
        sys.path.insert(0, _p)

import numpy as np

import concourse.bass as bass  # noqa: F401  (registers types)
from concourse import bacc
import concourse.tile as tile
import concourse.mybir as mybir
from concourse.bass_utils import run_bass_kernel_spmd

F32 = mybir.dt.float32
F32R = mybir.dt.float32r
AF = mybir.ActivationFunctionType
AX = mybir.AxisListType

N_CORES = 8
N_SLOTS = 8
B = 64
C_IN = 40
T_FULL = 8192
NEG = -1.0e30


def _chain(t0):
    """Per-layer valid/capacity length chain (mirrors the reference)."""
    t1 = (t0 - 10) // 2 + 1
    t2 = (t1 - 5) // 2 + 1
    t3 = (t2 - 5) // 2 + 1
    t4 = (t3 - 5) // 2 + 1
    t4p = t4 // 2
    t5 = (t4p - 5) // 2 + 1
    t5p = t5 // 2
    t6 = (t5p - 5) // 2 + 1
    t7 = (t6 - 3) // 2 + 1
    return t1, t2, t3, t4, t4p, t5, t5p, t6, t7


def _uniform_tail(caps):
    T4P0 = _chain(caps[0])[4]
    T5u = (T4P0 - 5) // 2 + 1
    T5pu = T5u // 2
    T6u = (T5pu - 5) // 2 + 1
    T7u = (T6u - 3) // 2 + 1
    return T4P0, T5u, T5pu, T6u, T7u


def _build_program(caps):
    """Emit the SPMD Bass/Tile program for the given 8 slot capacities."""
    T0 = list(caps)
    T1, T2, T3, T4, T4p = [], [], [], [], []
    for t0 in T0:
        t1, t2, t3, t4, t4p, _, _, _, _ = _chain(t0)
        T1.append(t1)
        T2.append(t2)
        T3.append(t3)
        T4.append(t4)
        T4p.append(t4p)
    T4P0, T5u, T5pu, T6u, T7u = _uniform_tail(caps)
    # phase-split widths of the pooled L4 output (uniform cat geometry)
    P4E = (T4P0 + 1) // 2
    P4O = T4P0 // 2
    P5E = (T5pu + 1) // 2
    P5O = T5pu // 2
    P6E = (T6u + 1) // 2
    P6O = T6u // 2

    nc = bacc.Bacc("TRN2", target_bir_lowering=False, debug=False)

    xs = [
        nc.dram_tensor(f"x{j}", [80, T0[j] // 2], F32R, kind="ExternalInput")
        for j in range(N_SLOTS)
    ]
    w1_t = nc.dram_tensor("w1s", [80, 5 * 96], F32R, kind="ExternalInput")
    wl_t = {
        l: nc.dram_tensor(f"w{l}s", [96, 5 * 96], F32R, kind="ExternalInput")
        for l in (2, 3, 4, 5, 6)
    }
    w7_t = nc.dram_tensor("w7s", [96, 3 * 128], F32R, kind="ExternalInput")
    lw1_t = nc.dram_tensor("lw1T", [128, 128], F32R, kind="ExternalInput")
    lw2_t = nc.dram_tensor("lw2T", [128, 64], F32R, kind="ExternalInput")
    lw3_t = nc.dram_tensor("lw3T", [64, 5], F32R, kind="ExternalInput")
    bias_t = nc.dram_tensor("biases", [128, 10], F32, kind="ExternalInput")
    fmask_t = nc.dram_tensor("fmask", [128, N_SLOTS * T7u], F32, kind="ExternalInput")
    out_t = nc.dram_tensor("out", [5, N_SLOTS], F32, kind="ExternalOutput")

    with tile.TileContext(nc) as tc:
        with (
            tc.tile_pool(name="wp", bufs=1) as wp,
            tc.tile_pool(name="xp", bufs=3) as xp,
            tc.tile_pool(name="actp", bufs=2) as ap_,
            tc.tile_pool(name="catp", bufs=1) as cp,
            tc.tile_pool(name="psp", bufs=2, space="PSUM") as pp,
        ):
            # ---- slot-0/1 inputs first (they gate the first matmuls) ----
            x2t = [None] * N_SLOTS

            def emit_load(j):
                th = T0[j] // 2
                x2 = xp.tile([80, th], F32R, tag="x2", name=f"x2_{j}")
                qn = (th + 3) // 4
                for q in range(4):
                    lo = q * qn
                    hi = min(th, lo + qn)
                    if lo < hi:
                        nc.sync.dma_start(x2[:, lo:hi], xs[j][:, lo:hi])
                x2t[j] = x2

            emit_load(7)
            emit_load(6)

            # ---- resident weights / constants (scalar HWDGE ring) ----
            w1s = wp.tile([80, 5 * 96], F32R, tag="w1")
            nc.scalar.dma_start(w1s[:], w1_t[:])
            wls = {}
            for l in (2, 3, 4, 5, 6):
                wls[l] = wp.tile([96, 5 * 96], F32R, tag=f"w{l}", name=f"w{l}sb")
                nc.scalar.dma_start(wls[l][:], wl_t[l][:])
            w7s = wp.tile([96, 3 * 128], F32R, tag="w7")
            nc.scalar.dma_start(w7s[:], w7_t[:])
            lw1s = wp.tile([128, 128], F32R, tag="lw1")
            nc.scalar.dma_start(lw1s[:], lw1_t[:])
            lw2s = wp.tile([128, 64], F32R, tag="lw2")
            nc.scalar.dma_start(lw2s[:], lw2_t[:])
            lw3s = wp.tile([64, 5], F32R, tag="lw3")
            nc.scalar.dma_start(lw3s[:], lw3_t[:])
            bs = wp.tile([128, 10], F32, tag="bias")
            nc.scalar.dma_start(bs[:], bias_t[:])
            fms = wp.tile([128, N_SLOTS * T7u], F32, tag="fmask")
            nc.scalar.dma_start(fms[:], fmask_t[:])

            # ---- PE warm-up: keep the HAM activity window busy while the
            # first x tiles stream in (outputs are never read) ----
            wtile = ap_.tile([80, 512], F32R, tag="wtile")
            nc.gpsimd.memset(wtile[:].bitcast(F32), 0.0)
            ps_w = pp.tile([96, 480], F32, tag="conv")
            for wi in range(14):
                nc.tensor.matmul(
                    ps_w[0:96, 0:480],
                    wtile[0:80, 0:96],
                    wtile[0:80, 0:480],
                    start=True,
                    stop=True,
                )

            # ---- concatenated slot-interleaved tail buffers (phase-split) ----
            p4Ecat = cp.tile([96, N_SLOTS * P4E], F32R, tag="p4Ecat")
            p4Ocat = cp.tile([96, N_SLOTS * P4O], F32R, tag="p4Ocat")
            nc.gpsimd.memset(p4Ecat[:].bitcast(F32), 0.0)
            nc.gpsimd.memset(p4Ocat[:].bitcast(F32), 0.0)
            p5Ecat = cp.tile([96, N_SLOTS * P5E], F32R, tag="p5Ecat")
            p5Ocat = cp.tile([96, N_SLOTS * P5O], F32R, tag="p5Ocat")
            b6Ecat = cp.tile([96, N_SLOTS * P6E], F32R, tag="b6Ecat")
            b6Ocat = cp.tile([96, N_SLOTS * P6O], F32R, tag="b6Ocat")
            b7cat = cp.tile([128, N_SLOTS * T7u], F32, tag="b7cat")

            def act(dst_ap, src_ap, bias_col, func=AF.Relu, prange=96):
                nc.scalar.activation(
                    dst_ap, src_ap, func, bias=bs[0:prange, bias_col : bias_col + 1]
                )

            # per-slot phase-split activation buffers
            bufEt = {}
            bufOt = {}

            def phase_evict(bEO, half, ts_, cols, ps, bias_col):
                # one ACT: in [96, 2, cols/2] (phase, t) <- psum pairs;
                # out col = phase*half + ts_/2 + t
                h = cols // 2
                inv = ps[0:96, 0:cols].rearrange("p (t q) -> p q t", q=2)
                outv = bEO[0:96, 0 : 2 * half].rearrange(
                    "p (q t) -> p q t", q=2)[:, :, ts_ // 2 : ts_ // 2 + h]
                act(outv, inv, bias_col)

            def emit_l1(j):
                x2 = x2t[j]
                t1 = T1[j]
                bEO = ap_.tile([96, t1], F32R, tag="b1", name=f"b1_{j}")
                half = t1 // 2
                for ts_ in range(0, t1, 2048):
                    cols = min(2048, t1 - ts_)
                    ps = pp.tile([96, 2048], F32, tag="conv", name=f"ps1_{j}_{ts_}")
                    for g in range(5):
                        lhsT = w1s[:, 96 * g : 96 * (g + 1)]
                        for cs in range(0, cols, 512):
                            cn = min(512, cols - cs)
                            t_off = ts_ + cs
                            nc.tensor.matmul(
                                ps[0:96, cs : cs + cn],
                                lhsT,
                                x2[0:80, t_off + g : t_off + g + cn],
                                start=(g == 0),
                                stop=(g == 4),
                            )
                    phase_evict(bEO, half, ts_, cols, ps, 0)
                bufEt[(1, j)] = bEO

            # tap order for K=5 on phase-split input: (E,0),(O,0),(E,1),(O,1),(E,2)
            PHASES5 = ((0, 0), (1, 0), (0, 1), (1, 1), (0, 2))

            def emit_conv(j, lsrc, ldst, w_sb, tin_all, tout_all, bias_col):
                tout = tout_all[j]
                srcEO = bufEt[(lsrc, j)]
                shalf = tin_all[j] // 2
                bEO = ap_.tile([96, tout], F32R, tag=f"b{ldst}", name=f"b{ldst}_{j}")
                half = tout // 2
                for ts_ in range(0, tout, 2048):
                    cols = min(2048, tout - ts_)
                    ps = pp.tile([96, 2048], F32, tag="conv", name=f"psc{ldst}_{j}_{ts_}")
                    for g, (ph, d) in enumerate(PHASES5):
                        lhsT = w_sb[:, 96 * g : 96 * (g + 1)]
                        off = ph * shalf + d
                        for cs in range(0, cols, 512):
                            cn = min(512, cols - cs)
                            t_off = ts_ + cs
                            nc.tensor.matmul(
                                ps[0:96, cs : cs + cn],
                                lhsT,
                                srcEO[0:96, off + t_off : off + t_off + cn],
                                start=(g == 0),
                                stop=(g == 4),
                            )
                    phase_evict(bEO, half, ts_, cols, ps, bias_col)
                bufEt[(ldst, j)] = bEO

            def emit_l4pool(j):
                t4 = T4[j]
                t4p = T4p[j]
                srcEO = bufEt[(3, j)]
                shalf = T3[j] // 2
                ps = pp.tile([96, 512], F32, tag="conv", name=f"ps4_{j}")
                for g, (ph, d) in enumerate(PHASES5):
                    off = ph * shalf + d
                    nc.tensor.matmul(
                        ps[0:96, 0:t4],
                        wls[4][:, 96 * g : 96 * (g + 1)],
                        srcEO[0:96, off : off + t4],
                        start=(g == 0),
                        stop=(g == 4),
                    )
                # pool pairs are exactly (even, odd) psum columns
                tE = ap_.tile([96, t4p], F32, tag="t4e", name=f"t4e_{j}")
                tO = ap_.tile([96, t4p], F32, tag="t4o", name=f"t4o_{j}")
                nc.vector.tensor_copy(tE[:, 0:t4p], ps[0:96, 0 : 2 * t4p - 1 : 2])
                nc.vector.tensor_copy(tO[:, 0:t4p], ps[0:96, 1 : 2 * t4p : 2])
                t4s = ap_.tile([96, t4p], F32, tag="t4s", name=f"t4s_{j}")
                nc.vector.tensor_add(t4s[:, 0:t4p], tE[0:96, 0:t4p], tO[0:96, 0:t4p])
                # relu(e+o+2*b4), phase-split into slot-interleaved cat buffers
                nE = (t4p + 1) // 2
                nO = t4p // 2
                act(p4Ecat[0:96, j : 8 * (nE - 1) + j + 1 : 8],
                    t4s[0:96, 0 : 2 * nE - 1 : 2], 3)
                if nO:
                    act(p4Ocat[0:96, j : 8 * (nO - 1) + j + 1 : 8],
                        t4s[0:96, 1 : 2 * nO : 2], 3)

            # ---- batched tail layers, emitted per 4-slot half ----
            def emit_tail_half(s0):
                p4Ev = p4Ecat[0:96, :].rearrange("p (t s) -> p t s", s=N_SLOTS)
                p4Ov = p4Ocat[0:96, :].rearrange("p (t s) -> p t s", s=N_SLOTS)
                ps5 = pp.tile([96, 4 * T5u], F32, tag="conv", name=f"ps5_{s0}")
                for g, (ph, d) in enumerate(PHASES5):
                    src = p4Ov if ph else p4Ev
                    nc.tensor.matmul(
                        ps5[0:96, :],
                        wls[5][:, 96 * g : 96 * (g + 1)],
                        src[:, d : d + T5u, s0 : s0 + 4],
                        start=(g == 0),
                        stop=(g == 4),
                    )
                buf5 = ap_.tile([96, 4 * T5u], F32, tag="b5", name=f"b5_{s0}")
                nc.vector.tensor_copy(buf5[:], ps5[0:96, :])
                b5v = buf5[:].rearrange("p (t s) -> p t s", s=4)
                tmp5 = ap_.tile([96, 4 * T5pu], F32, tag="t5", name=f"t5_{s0}")
                nc.vector.tensor_add(
                    tmp5[:].rearrange("p (t s) -> p t s", s=4),
                    b5v[:, 0 : 2 * T5pu : 2, :],
                    b5v[:, 1 : 2 * T5pu : 2, :],
                )
                t5v = tmp5[:].rearrange("p (u s) -> p u s", s=4)
                act(p5Ecat[0:96, :].rearrange("p (u s) -> p u s", s=N_SLOTS)[:, 0:P5E, s0 : s0 + 4],
                    t5v[:, 0 : 2 * P5E - 1 : 2, :], 4)
                act(p5Ocat[0:96, :].rearrange("p (u s) -> p u s", s=N_SLOTS)[:, 0:P5O, s0 : s0 + 4],
                    t5v[:, 1 : 2 * P5O : 2, :], 4)

                p5Ev = p5Ecat[0:96, :].rearrange("p (u s) -> p u s", s=N_SLOTS)
                p5Ov = p5Ocat[0:96, :].rearrange("p (u s) -> p u s", s=N_SLOTS)
                ps6 = pp.tile([96, 4 * T6u], F32, tag="conv", name=f"ps6_{s0}")
                for g, (ph, d) in enumerate(PHASES5):
                    src = p5Ov if ph else p5Ev
                    nc.tensor.matmul(
                        ps6[0:96, :],
                        wls[6][:, 96 * g : 96 * (g + 1)],
                        src[:, d : d + T6u, s0 : s0 + 4],
                        start=(g == 0),
                        stop=(g == 4),
                    )
                ps6v = ps6[0:96, 0 : 4 * T6u].rearrange("p (t s) -> p t s", s=4)
                act(b6Ecat[0:96, :].rearrange("p (v s) -> p v s", s=N_SLOTS)[:, 0:P6E, s0 : s0 + 4],
                    ps6v[:, 0 : 2 * P6E - 1 : 2, :], 5)
                act(b6Ocat[0:96, :].rearrange("p (v s) -> p v s", s=N_SLOTS)[:, 0:P6O, s0 : s0 + 4],
                    ps6v[:, 1 : 2 * P6O : 2, :], 5)

                b6Ev = b6Ecat[0:96, :].rearrange("p (v s) -> p v s", s=N_SLOTS)
                b6Ov = b6Ocat[0:96, :].rearrange("p (v s) -> p v s", s=N_SLOTS)
                ps7 = pp.tile([128, 4 * T7u], F32, tag="conv", name=f"ps7_{s0}")
                for g, (src, d) in enumerate(((b6Ev, 0), (b6Ov, 0), (b6Ev, 1))):
                    nc.tensor.matmul(
                        ps7[0:128, :],
                        w7s[:, 128 * g : 128 * (g + 1)],
                        src[:, d : d + T7u, s0 : s0 + 4],
                        start=(g == 0),
                        stop=(g == 2),
                    )
                act(b7cat[0:128, :].rearrange("p (t s) -> p t s", s=N_SLOTS)[:, 0:T7u, s0 : s0 + 4],
                    ps7[0:128, 0 : 4 * T7u].rearrange("p (t s) -> p t s", s=4),
                    6, prange=128)

            # ---- paired slot emission, smallest capacities first ----
            pair_order = [(7, 6), (5, 4), (3, 2), (1, 0)]
            for pi, (a, b) in enumerate(pair_order):
                emit_l1(a)
                emit_l1(b)
                if pi + 1 < len(pair_order):
                    emit_load(pair_order[pi + 1][0])
                    emit_load(pair_order[pi + 1][1])
                emit_conv(a, 1, 2, wls[2], T1, T2, 1)
                emit_conv(b, 1, 2, wls[2], T1, T2, 1)
                emit_conv(a, 2, 3, wls[3], T2, T3, 2)
                emit_conv(b, 2, 3, wls[3], T2, T3, 2)
                emit_l4pool(a)
                emit_l4pool(b)
                if (a, b) == (3, 2):
                    emit_tail_half(4)

            emit_tail_half(0)

            # ---- ragged masked max + MLP head ----
            tmpm = ap_.tile([128, N_SLOTS * T7u], F32, tag="tm")
            nc.vector.tensor_add(tmpm[:], b7cat[:], fms[:])
            xmax = ap_.tile([128, N_SLOTS], F32R, tag="xmax")
            nc.vector.reduce_max(
                xmax[:],
                tmpm[:].rearrange("p (t s) -> p s t", s=N_SLOTS),
                axis=AX.X,
            )

            psm1 = pp.tile([128, N_SLOTS], F32, tag="conv")
            nc.tensor.matmul(psm1[0:128, :], lw1s[:], xmax[:], start=True, stop=True)
            h1 = ap_.tile([128, N_SLOTS], F32R, tag="h1")
            act(h1[:], psm1[0:128, :], 7, prange=128)

            psm2 = pp.tile([64, N_SLOTS], F32, tag="conv")
            nc.tensor.matmul(psm2[0:64, :], lw2s[:], h1[:], start=True, stop=True)
            h2 = ap_.tile([64, N_SLOTS], F32R, tag="h2")
            act(h2[:], psm2[0:64, :], 8, prange=64)

            psm3 = pp.tile([5, N_SLOTS], F32, tag="conv")
            nc.tensor.matmul(psm3[0:5, :], lw3s[:], h2[0:64, :], start=True, stop=True)
            outsb = ap_.tile([5, N_SLOTS], F32, tag="osb")
            nc.vector.tensor_scalar_add(outsb[:], psm3[0:5, :], bs[0:5, 9:10])
            nc.sync.dma_start(out_t[:], outsb[:])

    nc.compile()
    return nc


def _prep_x(x, b, cap):
    """Host-side input re-layout: phase-major polyphase [80, cap//2]."""
    xb = np.asarray(x[b, :, :cap], np.float32)
    th = cap // 2
    return np.concatenate([xb[:, 0 : 2 * th : 2], xb[:, 1 : 2 * th : 2]], axis=0)


def _prep_weights(inp):
    """Host-side weight/bias re-layout (all tiny)."""
    w = {}
    w1 = np.asarray(inp["w1"], np.float32)  # [96, 40, 10]
    # polyphase phase-major rows (p*40+c), cols (m*96+o): W1[o, c, 2m+p]
    w["w1s"] = np.ascontiguousarray(
        w1.transpose(1, 2, 0).reshape(40, 5, 2, 96).transpose(2, 0, 1, 3).reshape(80, 480)
    )
    for l, scale in ((2, 1.0), (3, 1.0), (4, 1.0), (5, 0.5), (6, 0.5)):
        wl = np.asarray(inp[f"w{l}"], np.float32)  # [96, 96, 5]
        w[f"w{l}s"] = np.ascontiguousarray(wl.transpose(1, 2, 0).reshape(96, 480) * scale)
    w7 = np.asarray(inp["w7"], np.float32)  # [128, 96, 3]
    w["w7s"] = np.ascontiguousarray(w7.transpose(1, 2, 0).reshape(96, 384))
    w["lw1T"] = np.ascontiguousarray(np.asarray(inp["lw1"], np.float32).T)  # [128,128]
    w["lw2T"] = np.ascontiguousarray(np.asarray(inp["lw2"], np.float32).T)  # [128,64]
    w["lw3T"] = np.ascontiguousarray(np.asarray(inp["lw3"], np.float32).T)  # [64,5]

    biases = np.zeros((128, 10), np.float32)
    biases[0:96, 0] = np.asarray(inp["b1"], np.float32)
    biases[0:96, 1] = np.asarray(inp["b2"], np.float32)
    biases[0:96, 2] = np.asarray(inp["b3"], np.float32)
    biases[0:96, 3] = 2.0 * np.asarray(inp["b4"], np.float32)
    biases[0:96, 4] = 2.0 * np.asarray(inp["b5"], np.float32)
    biases[0:96, 5] = np.asarray(inp["b6"], np.float32)
    biases[0:128, 6] = np.asarray(inp["b7"], np.float32)
    biases[0:128, 7] = np.asarray(inp["lb1"], np.float32)
    biases[0:64, 8] = np.asarray(inp["lb2"], np.float32)
    biases[0:5, 9] = np.asarray(inp["lb3"], np.float32)
    w["biases"] = biases
    return w


def _schedule(len_mask):
    """Sort samples by length desc, deal round-robin: core c, slot j gets
    sample order[8j + c].  Slot capacity = rank-group max."""
    lens = np.asarray(len_mask, np.int64).clip(1, T_FULL)
    order = np.argsort(-lens, kind="stable")
    sample_of = np.zeros((N_CORES, N_SLOTS), np.int64)
    caps = []
    for j in range(N_SLOTS):
        grp = order[j * N_CORES : (j + 1) * N_CORES]
        for c in range(N_CORES):
            sample_of[c, j] = grp[c]
        cap = int(lens[grp].max())
        cap = max(cap, 1312)  # keep the whole chain >= 1 frame
        # round up to a multiple of 32 so T1..T4 are all even
        # (fp32r matmuls require an even moving-operand size)
        cap = min(((cap + 31) // 32) * 32, T_FULL)
        caps.append(cap)
    return order, sample_of, caps


def _make_in_maps(inputs, sample_of, caps):
    x = np.asarray(inputs["x_input"], np.float32)
    len_mask = np.asarray(inputs["len_mask"], np.int32)
    _, _, _, _, T7u = _uniform_tail(caps)
    w = _prep_weights(inputs)
    in_maps = []
    for c in range(N_CORES):
        m = dict(w)
        # slot-interleaved mask layout: column = t*8 + s
        fm2 = np.full((T7u, N_SLOTS), NEG, np.float32)
        for j in range(N_SLOTS):
            bidx = int(sample_of[c, j])
            m[f"x{j}"] = _prep_x(x, bidx, caps[j])
            lv7 = _chain(int(max(min(len_mask[bidx], T_FULL), 1312)))[8]
            lv7 = max(min(lv7, T7u), 1)
            fm2[0:lv7, j] = 0.0
        fmask = fm2.reshape(-1)
        m["fmask"] = np.ascontiguousarray(
            np.broadcast_to(fmask[None, :], (128, N_SLOTS * T7u))
        )
        in_maps.append(m)
    return in_maps


def _ensure_ntff_hook():
    """The agent image lacks ``antenv.axon_hooks``; seed a shim so
    ``run_bass_kernel_spmd(trace=True)`` can reach the axon NTFF profiler."""
    import types

    if "antenv.axon_hooks" in sys.modules:
        return
    try:
        from trn_agent_boot.trn_boot import _ntff_profile_via_ctypes

        hook = _ntff_profile_via_ctypes("/opt/axon/libaxon_pjrt.so")
    except Exception:
        hook = None
    mod = types.ModuleType("antenv.axon_hooks")
    state = {"hook": hook}
    mod.get_axon_ntff_profile_hook = lambda: state["hook"]
    mod.set_axon_ntff_profile_hook = lambda h: state.update(hook=h)
    sys.modules["antenv.axon_hooks"] = mod


_LDW_PATCHED = False


def _enable_ldw_opt():
    """Turn on walrus's LDWEIGHTS dedup (drops redundant weight reloads for
    back-to-back same-weight matmuls).  Verified bit-identical results."""
    global _LDW_PATCHED
    if _LDW_PATCHED:
        return
    try:
        import concourse.bass_utils as bu

        _orig = bu.run_command

        def run_command_ldw(argv, **kw):
            argv = [
                "--enable-ldw-opt=true" if a == "--enable-ldw-opt=false" else a
                for a in argv
            ]
            return _orig(argv, **kw)

        bu.run_command = run_command_ldw
        _LDW_PATCHED = True
    except Exception:
        pass


def _run(inputs, trace=False):
    if trace:
        _ensure_ntff_hook()
    _enable_ldw_opt()
    len_mask = np.asarray(inputs["len_mask"], np.int32)
    order, sample_of, caps = _schedule(len_mask)
    nc = _build_program(caps)
    in_maps = _make_in_maps(inputs, sample_of, caps)
    res = run_bass_kernel_spmd(
        nc, in_maps, core_ids=list(range(N_CORES)), trace=trace
    )
    out = np.zeros((B, 5), np.float32)
    for c in range(N_CORES):
        o = res.results[c]["out"]  # [5, 8]
        for j in range(N_SLOTS):
            out[int(sample_of[c, j])] = o[:, j]
    return out, res


def kernel(**inputs):
    out, _ = _run(inputs, trace=False)
    return out


# revision 27
# speedup vs baseline: 1.1297x; 1.1297x over previous
"""Trainium2 Bass kernel for the conv1d-stack + MLP ragged-sequence model.

Strategy:
  - Pure data parallelism over 8 NeuronCores: 8 samples per core.
  - Samples are sorted by valid length (descending) and dealt round-robin to
    cores, so every core runs an IDENTICAL (SPMD) program whose per-slot
    sizes are the per-rank-group maximum length (exploits raggedness).
  - Convs run on the TensorEngine as float32r matmuls with per-tap PSUM
    accumulation.  All activations are kept PHASE-SPLIT (even/odd time
    samples in separate buffers), which turns every stride-2 conv into
    stride-1 matmul reads — stride-2 moving operands run the PE at half
    rate, stride-1 reach the full warm rate (~0.45 ns/col measured).
  - Layers 5-7 additionally interleave all 8 slots (column = t*8 + s) so
    their matmul reads are single contiguous slabs.
  - Slots are emitted in pairs so PSUM-eviction latency of slot j hides
    behind slot j+1's matmuls.  Bias+ReLU eviction on ScalarE (which also
    does the phase split via strided reads), avgpool on VectorE.
  - The ragged masked-max uses a host-built additive -1e30 mask (a data
    input, SPMD-safe), then the 3-layer MLP runs on-device.
"""

import os
import sys

for _p in ("/opt/trn_rl_repo",):
    if _p not in sys.path and os.path.isdir(_p):
        sys.path.insert(0, _p)

import numpy as np

import concourse.bass as bass  # noqa: F401  (registers types)
from concourse import bacc
import concourse.tile as tile
import concourse.mybir as mybir
from concourse.bass_utils import run_bass_kernel_spmd

F32 = mybir.dt.float32
F32R = mybir.dt.float32r
AF = mybir.ActivationFunctionType
AX = mybir.AxisListType

N_CORES = 8
N_SLOTS = 8
B = 64
C_IN = 40
T_FULL = 8192
NEG = -1.0e30


def _chain(t0):
    """Per-layer valid/capacity length chain (mirrors the reference)."""
    t1 = (t0 - 10) // 2 + 1
    t2 = (t1 - 5) // 2 + 1
    t3 = (t2 - 5) // 2 + 1
    t4 = (t3 - 5) // 2 + 1
    t4p = t4 // 2
    t5 = (t4p - 5) // 2 + 1
    t5p = t5 // 2
    t6 = (t5p - 5) // 2 + 1
    t7 = (t6 - 3) // 2 + 1
    return t1, t2, t3, t4, t4p, t5, t5p, t6, t7


def _uniform_tail(caps):
    T4P0 = _chain(caps[0])[4]
    T5u = (T4P0 - 5) // 2 + 1
    T5pu = T5u // 2
    T6u = (T5pu - 5) // 2 + 1
    T7u = (T6u - 3) // 2 + 1
    return T4P0, T5u, T5pu, T6u, T7u


def _build_program(caps):
    """Emit the SPMD Bass/Tile program for the given 8 slot capacities."""
    T0 = list(caps)
    T1, T2, T3, T4, T4p = [], [], [], [], []
    for t0 in T0:
        t1, t2, t3, t4, t4p, _, _, _, _ = _chain(t0)
        T1.append(t1)
        T2.append(t2)
        T3.append(t3)
        T4.append(t4)
        T4p.append(t4p)
    T4P0, T5u, T5pu, T6u, T7u = _uniform_tail(caps)
    # phase-split widths of the pooled L4 output (uniform cat geometry)
    P4E = (T4P0 + 1) // 2
    P4O = T4P0 // 2
    P5E = (T5pu + 1) // 2
    P5O = T5pu // 2
    P6E = (T6u + 1) // 2
    P6O = T6u // 2

    nc = bacc.Bacc("TRN2", target_bir_lowering=False, debug=False)

    xs = [
        nc.dram_tensor(f"x{j}", [80, T0[j] // 2], F32R, kind="ExternalInput")
        for j in range(N_SLOTS)
    ]
    w1_t = nc.dram_tensor("w1s", [80, 5 * 96], F32R, kind="ExternalInput")
    wl_t = {
        l: nc.dram_tensor(f"w{l}s", [96, 5 * 96], F32R, kind="ExternalInput")
        for l in (2, 3, 4, 5, 6)
    }
    w7_t = nc.dram_tensor("w7s", [96, 3 * 128], F32R, kind="ExternalInput")
    lw1_t = nc.dram_tensor("lw1T", [128, 128], F32R, kind="ExternalInput")
    lw2_t = nc.dram_tensor("lw2T", [128, 64], F32R, kind="ExternalInput")
    lw3_t = nc.dram_tensor("lw3T", [64, 5], F32R, kind="ExternalInput")
    bias_t = nc.dram_tensor("biases", [128, 10], F32, kind="ExternalInput")
    fmask_t = nc.dram_tensor("fmask", [128, N_SLOTS * T7u], F32, kind="ExternalInput")
    out_t = nc.dram_tensor("out", [5, N_SLOTS], F32, kind="ExternalOutput")

    with tile.TileContext(nc) as tc:
        with (
            tc.tile_pool(name="wp", bufs=1) as wp,
            tc.tile_pool(name="xp", bufs=3) as xp,
            tc.tile_pool(name="actp", bufs=2) as ap_,
            tc.tile_pool(name="catp", bufs=1) as cp,
            tc.tile_pool(name="psp", bufs=2, space="PSUM") as pp,
        ):
            # ---- slot-0/1 inputs first (they gate the first matmuls) ----
            x2t = [None] * N_SLOTS

            def emit_load(j):
                th = T0[j] // 2
                x2 = xp.tile([80, th], F32R, tag="x2", name=f"x2_{j}")
                qn = (th + 3) // 4
                for q in range(4):
                    lo = q * qn
                    hi = min(th, lo + qn)
                    if lo < hi:
                        nc.sync.dma_start(x2[:, lo:hi], xs[j][:, lo:hi])
                x2t[j] = x2

            emit_load(7)
            emit_load(6)

            # ---- resident weights / constants (scalar HWDGE ring) ----
            w1s = wp.tile([80, 5 * 96], F32R, tag="w1")
            nc.scalar.dma_start(w1s[:], w1_t[:])
            wls = {}
            for l in (2, 3, 4, 5, 6):
                wls[l] = wp.tile([96, 5 * 96], F32R, tag=f"w{l}", name=f"w{l}sb")
                nc.scalar.dma_start(wls[l][:], wl_t[l][:])
            w7s = wp.tile([96, 3 * 128], F32R, tag="w7")
            nc.scalar.dma_start(w7s[:], w7_t[:])
            lw1s = wp.tile([128, 128], F32R, tag="lw1")
            nc.scalar.dma_start(lw1s[:], lw1_t[:])
            lw2s = wp.tile([128, 64], F32R, tag="lw2")
            nc.scalar.dma_start(lw2s[:], lw2_t[:])
            lw3s = wp.tile([64, 5], F32R, tag="lw3")
            nc.scalar.dma_start(lw3s[:], lw3_t[:])
            bs = wp.tile([128, 10], F32, tag="bias")
            nc.scalar.dma_start(bs[:], bias_t[:])
            fms = wp.tile([128, N_SLOTS * T7u], F32, tag="fmask")
            nc.scalar.dma_start(fms[:], fmask_t[:])

            # ---- PE warm-up: keep the HAM activity window busy while the
            # first x tiles stream in (outputs are never read) ----
            wtile = ap_.tile([80, 512], F32R, tag="wtile")
            nc.gpsimd.memset(wtile[:].bitcast(F32), 0.0)
            ps_w = pp.tile([96, 480], F32, tag="conv")
            for wi in range(14):
                nc.tensor.matmul(
                    ps_w[0:96, 0:480],
                    wtile[0:80, 0:96],
                    wtile[0:80, 0:480],
                    start=True,
                    stop=True,
                )

            # ---- concatenated slot-interleaved tail buffers (phase-split) ----
            p4Ecat = cp.tile([96, N_SLOTS * P4E], F32R, tag="p4Ecat")
            p4Ocat = cp.tile([96, N_SLOTS * P4O], F32R, tag="p4Ocat")
            nc.gpsimd.memset(p4Ecat[:].bitcast(F32), 0.0)
            nc.gpsimd.memset(p4Ocat[:].bitcast(F32), 0.0)
            p5Ecat = cp.tile([96, N_SLOTS * P5E], F32R, tag="p5Ecat")
            p5Ocat = cp.tile([96, N_SLOTS * P5O], F32R, tag="p5Ocat")
            b6Ecat = cp.tile([96, N_SLOTS * P6E], F32R, tag="b6Ecat")
            b6Ocat = cp.tile([96, N_SLOTS * P6O], F32R, tag="b6Ocat")
            b7cat = cp.tile([128, N_SLOTS * T7u], F32, tag="b7cat")

            def act(dst_ap, src_ap, bias_col, func=AF.Relu, prange=96):
                nc.scalar.activation(
                    dst_ap, src_ap, func, bias=bs[0:prange, bias_col : bias_col + 1]
                )

            # per-slot phase-split activation buffers
            bufEt = {}
            bufOt = {}

            def phase_evict(bEO, half, ts_, cols, ps, bias_col):
                # one ACT: in [96, 2, cols/2] (phase, t) <- psum pairs;
                # out col = phase*half + ts_/2 + t
                h = cols // 2
                inv = ps[0:96, 0:cols].rearrange("p (t q) -> p q t", q=2)
                outv = bEO[0:96, 0 : 2 * half].rearrange(
                    "p (q t) -> p q t", q=2)[:, :, ts_ // 2 : ts_ // 2 + h]
                act(outv, inv, bias_col)

            def emit_l1(j):
                x2 = x2t[j]
                t1 = T1[j]
                bEO = ap_.tile([96, t1], F32R, tag="b1", name=f"b1_{j}")
                half = t1 // 2
                for ts_ in range(0, t1, 2048):
                    cols = min(2048, t1 - ts_)
                    ps = pp.tile([96, 2048], F32, tag="conv", name=f"ps1_{j}_{ts_}")
                    for g in range(5):
                        lhsT = w1s[:, 96 * g : 96 * (g + 1)]
                        for cs in range(0, cols, 512):
                            cn = min(512, cols - cs)
                            t_off = ts_ + cs
                            nc.tensor.matmul(
                                ps[0:96, cs : cs + cn],
                                lhsT,
                                x2[0:80, t_off + g : t_off + g + cn],
                                start=(g == 0),
                                stop=(g == 4),
                            )
                    phase_evict(bEO, half, ts_, cols, ps, 0)
                bufEt[(1, j)] = bEO

            # tap order for K=5 on phase-split input: (E,0),(O,0),(E,1),(O,1),(E,2)
            PHASES5 = ((0, 0), (1, 0), (0, 1), (1, 1), (0, 2))

            def emit_conv(j, lsrc, ldst, w_sb, tin_all, tout_all, bias_col):
                tout = tout_all[j]
                srcEO = bufEt[(lsrc, j)]
                shalf = tin_all[j] // 2
                bEO = ap_.tile([96, tout], F32R, tag=f"b{ldst}", name=f"b{ldst}_{j}")
                half = tout // 2
                for ts_ in range(0, tout, 2048):
                    cols = min(2048, tout - ts_)
                    ps = pp.tile([96, 2048], F32, tag="conv", name=f"psc{ldst}_{j}_{ts_}")
                    for g, (ph, d) in enumerate(PHASES5):
                        lhsT = w_sb[:, 96 * g : 96 * (g + 1)]
                        off = ph * shalf + d
                        for cs in range(0, cols, 512):
                            cn = min(512, cols - cs)
                            t_off = ts_ + cs
                            nc.tensor.matmul(
                                ps[0:96, cs : cs + cn],
                                lhsT,
                                srcEO[0:96, off + t_off : off + t_off + cn],
                                start=(g == 0),
                                stop=(g == 4),
                            )
                    phase_evict(bEO, half, ts_, cols, ps, bias_col)
                bufEt[(ldst, j)] = bEO

            def emit_l4pool(j):
                t4 = T4[j]
                t4p = T4p[j]
                srcEO = bufEt[(3, j)]
                shalf = T3[j] // 2
                ps = pp.tile([96, 512], F32, tag="conv", name=f"ps4_{j}")
                for g, (ph, d) in enumerate(PHASES5):
                    off = ph * shalf + d
                    nc.tensor.matmul(
                        ps[0:96, 0:t4],
                        wls[4][:, 96 * g : 96 * (g + 1)],
                        srcEO[0:96, off : off + t4],
                        start=(g == 0),
                        stop=(g == 4),
                    )
                # pool pairs are exactly (even, odd) psum columns
                tE = ap_.tile([96, t4p], F32, tag="t4e", name=f"t4e_{j}")
                tO = ap_.tile([96, t4p], F32, tag="t4o", name=f"t4o_{j}")
                nc.vector.tensor_copy(tE[:, 0:t4p], ps[0:96, 0 : 2 * t4p - 1 : 2])
                nc.vector.tensor_copy(tO[:, 0:t4p], ps[0:96, 1 : 2 * t4p : 2])
                t4s = ap_.tile([96, t4p], F32, tag="t4s", name=f"t4s_{j}")
                nc.vector.tensor_add(t4s[:, 0:t4p], tE[0:96, 0:t4p], tO[0:96, 0:t4p])
                # relu(e+o+2*b4), phase-split into slot-interleaved cat buffers
                nE = (t4p + 1) // 2
                nO = t4p // 2
                act(p4Ecat[0:96, j : 8 * (nE - 1) + j + 1 : 8],
                    t4s[0:96, 0 : 2 * nE - 1 : 2], 3)
                if nO:
                    act(p4Ocat[0:96, j : 8 * (nO - 1) + j + 1 : 8],
                        t4s[0:96, 1 : 2 * nO : 2], 3)

            # ---- batched tail layers, emitted per 4-slot half ----
            def emit_tail_half(s0):
                p4Ev = p4Ecat[0:96, :].rearrange("p (t s) -> p t s", s=N_SLOTS)
                p4Ov = p4Ocat[0:96, :].rearrange("p (t s) -> p t s", s=N_SLOTS)
                ps5 = pp.tile([96, 4 * T5u], F32, tag="conv", name=f"ps5_{s0}")
                for g, (ph, d) in enumerate(PHASES5):
                    src = p4Ov if ph else p4Ev
                    nc.tensor.matmul(
                        ps5[0:96, :],
                        wls[5][:, 96 * g : 96 * (g + 1)],
                        src[:, d : d + T5u, s0 : s0 + 4],
                        start=(g == 0),
                        stop=(g == 4),
                    )
                buf5 = ap_.tile([96, 4 * T5u], F32, tag="b5", name=f"b5_{s0}")
                nc.vector.tensor_copy(buf5[:], ps5[0:96, :])
                b5v = buf5[:].rearrange("p (t s) -> p t s", s=4)
                tmp5 = ap_.tile([96, 4 * T5pu], F32, tag="t5", name=f"t5_{s0}")
                nc.vector.tensor_add(
                    tmp5[:].rearrange("p (t s) -> p t s", s=4),
                    b5v[:, 0 : 2 * T5pu : 2, :],
                    b5v[:, 1 : 2 * T5pu : 2, :],
                )
                t5v = tmp5[:].rearrange("p (u s) -> p u s", s=4)
                act(p5Ecat[0:96, :].rearrange("p (u s) -> p u s", s=N_SLOTS)[:, 0:P5E, s0 : s0 + 4],
                    t5v[:, 0 : 2 * P5E - 1 : 2, :], 4)
                act(p5Ocat[0:96, :].rearrange("p (u s) -> p u s", s=N_SLOTS)[:, 0:P5O, s0 : s0 + 4],
                    t5v[:, 1 : 2 * P5O : 2, :], 4)

                p5Ev = p5Ecat[0:96, :].rearrange("p (u s) -> p u s", s=N_SLOTS)
                p5Ov = p5Ocat[0:96, :].rearrange("p (u s) -> p u s", s=N_SLOTS)
                ps6 = pp.tile([96, 4 * T6u], F32, tag="conv", name=f"ps6_{s0}")
                for g, (ph, d) in enumerate(PHASES5):
                    src = p5Ov if ph else p5Ev
                    nc.tensor.matmul(
                        ps6[0:96, :],
                        wls[6][:, 96 * g : 96 * (g + 1)],
                        src[:, d : d + T6u, s0 : s0 + 4],
                        start=(g == 0),
                        stop=(g == 4),
                    )
                ps6v = ps6[0:96, 0 : 4 * T6u].rearrange("p (t s) -> p t s", s=4)
                act(b6Ecat[0:96, :].rearrange("p (v s) -> p v s", s=N_SLOTS)[:, 0:P6E, s0 : s0 + 4],
                    ps6v[:, 0 : 2 * P6E - 1 : 2, :], 5)
                act(b6Ocat[0:96, :].rearrange("p (v s) -> p v s", s=N_SLOTS)[:, 0:P6O, s0 : s0 + 4],
                    ps6v[:, 1 : 2 * P6O : 2, :], 5)

                b6Ev = b6Ecat[0:96, :].rearrange("p (v s) -> p v s", s=N_SLOTS)
                b6Ov = b6Ocat[0:96, :].rearrange("p (v s) -> p v s", s=N_SLOTS)
                ps7 = pp.tile([128, 4 * T7u], F32, tag="conv", name=f"ps7_{s0}")
                for g, (src, d) in enumerate(((b6Ev, 0), (b6Ov, 0), (b6Ev, 1))):
                    nc.tensor.matmul(
                        ps7[0:128, :],
                        w7s[:, 128 * g : 128 * (g + 1)],
                        src[:, d : d + T7u, s0 : s0 + 4],
                        start=(g == 0),
                        stop=(g == 2),
                    )
                act(b7cat[0:128, :].rearrange("p (t s) -> p t s", s=N_SLOTS)[:, 0:T7u, s0 : s0 + 4],
                    ps7[0:128, 0 : 4 * T7u].rearrange("p (t s) -> p t s", s=4),
                    6, prange=128)

            # ---- paired slot emission, smallest capacities first ----
            pair_order = [(7, 6), (5, 4), (3, 2), (1, 0)]
            for pi, (a, b) in enumerate(pair_order):
                emit_l1(a)
                emit_l1(b)
                if pi + 1 < len(pair_order):
                    emit_load(pair_order[pi + 1][0])
                    emit_load(pair_order[pi + 1][1])
                emit_conv(a, 1, 2, wls[2], T1, T2, 1)
                emit_conv(b, 1, 2, wls[2], T1, T2, 1)
                emit_conv(a, 2, 3, wls[3], T2, T3, 2)
                emit_conv(b, 2, 3, wls[3], T2, T3, 2)
                emit_l4pool(a)
                emit_l4pool(b)

            emit_tail_half(4)
            emit_tail_half(0)

            # ---- ragged masked max + MLP head ----
            tmpm = ap_.tile([128, N_SLOTS * T7u], F32, tag="tm")
            nc.vector.tensor_add(tmpm[:], b7cat[:], fms[:])
            xmax = ap_.tile([128, N_SLOTS], F32R, tag="xmax")
            nc.vector.reduce_max(
                xmax[:],
                tmpm[:].rearrange("p (t s) -> p s t", s=N_SLOTS),
                axis=AX.X,
            )

            psm1 = pp.tile([128, N_SLOTS], F32, tag="conv")
            nc.tensor.matmul(psm1[0:128, :], lw1s[:], xmax[:], start=True, stop=True)
            h1 = ap_.tile([128, N_SLOTS], F32R, tag="h1")
            act(h1[:], psm1[0:128, :], 7, prange=128)

            psm2 = pp.tile([64, N_SLOTS], F32, tag="conv")
            nc.tensor.matmul(psm2[0:64, :], lw2s[:], h1[:], start=True, stop=True)
            h2 = ap_.tile([64, N_SLOTS], F32R, tag="h2")
            act(h2[:], psm2[0:64, :], 8, prange=64)

            psm3 = pp.tile([5, N_SLOTS], F32, tag="conv")
            nc.tensor.matmul(psm3[0:5, :], lw3s[:], h2[0:64, :], start=True, stop=True)
            outsb = ap_.tile([5, N_SLOTS], F32, tag="osb")
            nc.vector.tensor_scalar_add(outsb[:], psm3[0:5, :], bs[0:5, 9:10])
            nc.sync.dma_start(out_t[:], outsb[:])

    nc.compile()
    return nc


def _prep_x(x, b, cap):
    """Host-side input re-layout: phase-major polyphase [80, cap//2]."""
    xb = np.asarray(x[b, :, :cap], np.float32)
    th = cap // 2
    return np.concatenate([xb[:, 0 : 2 * th : 2], xb[:, 1 : 2 * th : 2]], axis=0)


def _prep_weights(inp):
    """Host-side weight/bias re-layout (all tiny)."""
    w = {}
    w1 = np.asarray(inp["w1"], np.float32)  # [96, 40, 10]
    # polyphase phase-major rows (p*40+c), cols (m*96+o): W1[o, c, 2m+p]
    w["w1s"] = np.ascontiguousarray(
        w1.transpose(1, 2, 0).reshape(40, 5, 2, 96).transpose(2, 0, 1, 3).reshape(80, 480)
    )
    for l, scale in ((2, 1.0), (3, 1.0), (4, 1.0), (5, 0.5), (6, 0.5)):
        wl = np.asarray(inp[f"w{l}"], np.float32)  # [96, 96, 5]
        w[f"w{l}s"] = np.ascontiguousarray(wl.transpose(1, 2, 0).reshape(96, 480) * scale)
    w7 = np.asarray(inp["w7"], np.float32)  # [128, 96, 3]
    w["w7s"] = np.ascontiguousarray(w7.transpose(1, 2, 0).reshape(96, 384))
    w["lw1T"] = np.ascontiguousarray(np.asarray(inp["lw1"], np.float32).T)  # [128,128]
    w["lw2T"] = np.ascontiguousarray(np.asarray(inp["lw2"], np.float32).T)  # [128,64]
    w["lw3T"] = np.ascontiguousarray(np.asarray(inp["lw3"], np.float32).T)  # [64,5]

    biases = np.zeros((128, 10), np.float32)
    biases[0:96, 0] = np.asarray(inp["b1"], np.float32)
    biases[0:96, 1] = np.asarray(inp["b2"], np.float32)
    biases[0:96, 2] = np.asarray(inp["b3"], np.float32)
    biases[0:96, 3] = 2.0 * np.asarray(inp["b4"], np.float32)
    biases[0:96, 4] = 2.0 * np.asarray(inp["b5"], np.float32)
    biases[0:96, 5] = np.asarray(inp["b6"], np.float32)
    biases[0:128, 6] = np.asarray(inp["b7"], np.float32)
    biases[0:128, 7] = np.asarray(inp["lb1"], np.float32)
    biases[0:64, 8] = np.asarray(inp["lb2"], np.float32)
    biases[0:5, 9] = np.asarray(inp["lb3"], np.float32)
    w["biases"] = biases
    return w


def _schedule(len_mask):
    """Sort samples by length desc, deal round-robin: core c, slot j gets
    sample order[8j + c].  Slot capacity = rank-group max."""
    lens = np.asarray(len_mask, np.int64).clip(1, T_FULL)
    order = np.argsort(-lens, kind="stable")
    sample_of = np.zeros((N_CORES, N_SLOTS), np.int64)
    caps = []
    for j in range(N_SLOTS):
        grp = order[j * N_CORES : (j + 1) * N_CORES]
        for c in range(N_CORES):
            sample_of[c, j] = grp[c]
        cap = int(lens[grp].max())
        cap = max(cap, 1312)  # keep the whole chain >= 1 frame
        # round up to a multiple of 32 so T1..T4 are all even
        # (fp32r matmuls require an even moving-operand size)
        cap = min(((cap + 31) // 32) * 32, T_FULL)
        caps.append(cap)
    return order, sample_of, caps


def _make_in_maps(inputs, sample_of, caps):
    x = np.asarray(inputs["x_input"], np.float32)
    len_mask = np.asarray(inputs["len_mask"], np.int32)
    _, _, _, _, T7u = _uniform_tail(caps)
    w = _prep_weights(inputs)
    in_maps = []
    for c in range(N_CORES):
        m = dict(w)
        # slot-interleaved mask layout: column = t*8 + s
        fm2 = np.full((T7u, N_SLOTS), NEG, np.float32)
        for j in range(N_SLOTS):
            bidx = int(sample_of[c, j])
            m[f"x{j}"] = _prep_x(x, bidx, caps[j])
            lv7 = _chain(int(max(min(len_mask[bidx], T_FULL), 1312)))[8]
            lv7 = max(min(lv7, T7u), 1)
            fm2[0:lv7, j] = 0.0
        fmask = fm2.reshape(-1)
        m["fmask"] = np.ascontiguousarray(
            np.broadcast_to(fmask[None, :], (128, N_SLOTS * T7u))
        )
        in_maps.append(m)
    return in_maps


def _ensure_ntff_hook():
    """The agent image lacks ``antenv.axon_hooks``; seed a shim so
    ``run_bass_kernel_spmd(trace=True)`` can reach the axon NTFF profiler."""
    import types

    if "antenv.axon_hooks" in sys.modules:
        return
    try:
        from trn_agent_boot.trn_boot import _ntff_profile_via_ctypes

        hook = _ntff_profile_via_ctypes("/opt/axon/libaxon_pjrt.so")
    except Exception:
        hook = None
    mod = types.ModuleType("antenv.axon_hooks")
    state = {"hook": hook}
    mod.get_axon_ntff_profile_hook = lambda: state["hook"]
    mod.set_axon_ntff_profile_hook = lambda h: state.update(hook=h)
    sys.modules["antenv.axon_hooks"] = mod


_LDW_PATCHED = False


def _enable_ldw_opt():
    """Turn on walrus's LDWEIGHTS dedup (drops redundant weight reloads for
    back-to-back same-weight matmuls).  Verified bit-identical results."""
    global _LDW_PATCHED
    if _LDW_PATCHED:
        return
    try:
        import concourse.bass_utils as bu

        _orig = bu.run_command

        def run_command_ldw(argv, **kw):
            argv = [
                "--enable-ldw-opt=true" if a == "--enable-ldw-opt=false" else a
                for a in argv
            ]
            return _orig(argv, **kw)

        bu.run_command = run_command_ldw
        _LDW_PATCHED = True
    except Exception:
        pass


def _run(inputs, trace=False):
    if trace:
        _ensure_ntff_hook()
    _enable_ldw_opt()
    len_mask = np.asarray(inputs["len_mask"], np.int32)
    order, sample_of, caps = _schedule(len_mask)
    nc = _build_program(caps)
    in_maps = _make_in_maps(inputs, sample_of, caps)
    res = run_bass_kernel_spmd(
        nc, in_maps, core_ids=list(range(N_CORES)), trace=trace
    )
    out = np.zeros((B, 5), np.float32)
    for c in range(N_CORES):
        o = res.results[c]["out"]  # [5, 8]
        for j in range(N_SLOTS):
            out[int(sample_of[c, j])] = o[:, j]
    return out, res


def kernel(**inputs):
    out, _ = _run(inputs, trace=False)
    return out


# revision 29
# speedup vs baseline: 1.1653x; 1.0315x over previous
"""Trainium2 Bass kernel for the conv1d-stack + MLP ragged-sequence model.

Strategy:
  - Pure data parallelism over 8 NeuronCores: 8 samples per core.
  - Samples are sorted by valid length (descending) and dealt round-robin to
    cores, so every core runs an IDENTICAL (SPMD) program whose per-slot
    sizes are the per-rank-group maximum length (exploits raggedness).
  - Convs run on the TensorEngine as float32r matmuls with per-tap PSUM
    accumulation.  All activations are kept PHASE-SPLIT (even/odd time
    samples in separate buffers), which turns every stride-2 conv into
    stride-1 matmul reads — stride-2 moving operands run the PE at half
    rate, stride-1 reach the full warm rate (~0.45 ns/col measured).
  - Layers 5-7 additionally interleave all 8 slots (column = t*8 + s) so
    their matmul reads are single contiguous slabs.
  - Slots are emitted in pairs so PSUM-eviction latency of slot j hides
    behind slot j+1's matmuls.  Bias+ReLU eviction on ScalarE (which also
    does the phase split via strided reads), avgpool on VectorE.
  - The ragged masked-max uses a host-built additive -1e30 mask (a data
    input, SPMD-safe), then the 3-layer MLP runs on-device.
"""

import os
import sys

for _p in ("/opt/trn_rl_repo",):
    if _p not in sys.path and os.path.isdir(_p):
        sys.path.insert(0, _p)

import numpy as np

import concourse.bass as bass  # noqa: F401  (registers types)
from concourse import bacc
import concourse.tile as tile
import concourse.mybir as mybir
from concourse.bass_utils import run_bass_kernel_spmd

F32 = mybir.dt.float32
F32R = mybir.dt.float32r
AF = mybir.ActivationFunctionType
AX = mybir.AxisListType

N_CORES = 8
N_SLOTS = 8
B = 64
C_IN = 40
T_FULL = 8192
NEG = -1.0e30


def _chain(t0):
    """Per-layer valid/capacity length chain (mirrors the reference)."""
    t1 = (t0 - 10) // 2 + 1
    t2 = (t1 - 5) // 2 + 1
    t3 = (t2 - 5) // 2 + 1
    t4 = (t3 - 5) // 2 + 1
    t4p = t4 // 2
    t5 = (t4p - 5) // 2 + 1
    t5p = t5 // 2
    t6 = (t5p - 5) // 2 + 1
    t7 = (t6 - 3) // 2 + 1
    return t1, t2, t3, t4, t4p, t5, t5p, t6, t7


def _uniform_tail(caps):
    T4P0 = _chain(caps[0])[4]
    T5u = (T4P0 - 5) // 2 + 1
    T5pu = T5u // 2
    T6u = (T5pu - 5) // 2 + 1
    T7u = (T6u - 3) // 2 + 1
    return T4P0, T5u, T5pu, T6u, T7u


def _build_program(caps):
    """Emit the SPMD Bass/Tile program for the given 8 slot capacities."""
    T0 = list(caps)
    T1, T2, T3, T4, T4p = [], [], [], [], []
    for t0 in T0:
        t1, t2, t3, t4, t4p, _, _, _, _ = _chain(t0)
        T1.append(t1)
        T2.append(t2)
        T3.append(t3)
        T4.append(t4)
        T4p.append(t4p)
    T4P0, T5u, T5pu, T6u, T7u = _uniform_tail(caps)
    # phase-split widths of the pooled L4 output (uniform cat geometry)
    P4E = (T4P0 + 1) // 2
    P4O = T4P0 // 2
    P5E = (T5pu + 1) // 2
    P5O = T5pu // 2
    P6E = (T6u + 1) // 2
    P6O = T6u // 2

    nc = bacc.Bacc("TRN2", target_bir_lowering=False, debug=False)

    xs = [
        nc.dram_tensor(f"x{j}", [80, T0[j] // 2], F32R, kind="ExternalInput")
        for j in range(N_SLOTS)
    ]
    w1_t = nc.dram_tensor("w1s", [80, 5 * 96], F32R, kind="ExternalInput")
    wl_t = {
        l: nc.dram_tensor(f"w{l}s", [96, 5 * 96], F32R, kind="ExternalInput")
        for l in (2, 3, 4, 5, 6)
    }
    w7_t = nc.dram_tensor("w7s", [96, 3 * 128], F32R, kind="ExternalInput")
    lw1_t = nc.dram_tensor("lw1T", [128, 128], F32R, kind="ExternalInput")
    lw2_t = nc.dram_tensor("lw2T", [128, 64], F32R, kind="ExternalInput")
    lw3_t = nc.dram_tensor("lw3T", [64, 5], F32R, kind="ExternalInput")
    bias_t = nc.dram_tensor("biases", [128, 10], F32, kind="ExternalInput")
    fmask_t = nc.dram_tensor("fmask", [128, N_SLOTS * T7u], F32, kind="ExternalInput")
    out_t = nc.dram_tensor("out", [5, N_SLOTS], F32, kind="ExternalOutput")

    with tile.TileContext(nc) as tc:
        with (
            tc.tile_pool(name="wp", bufs=1) as wp,
            tc.tile_pool(name="xp", bufs=3) as xp,
            tc.tile_pool(name="actp", bufs=2) as ap_,
            tc.tile_pool(name="catp", bufs=1) as cp,
            tc.tile_pool(name="psp", bufs=2, space="PSUM") as pp,
        ):
            # ---- slot-0/1 inputs first (they gate the first matmuls) ----
            x2t = [None] * N_SLOTS

            def emit_load(j):
                th = T0[j] // 2
                x2 = xp.tile([80, th], F32R, tag="x2", name=f"x2_{j}")
                qn = (th + 3) // 4
                for q in range(4):
                    lo = q * qn
                    hi = min(th, lo + qn)
                    if lo < hi:
                        nc.sync.dma_start(x2[:, lo:hi], xs[j][:, lo:hi])
                x2t[j] = x2

            emit_load(7)
            emit_load(6)

            # ---- resident weights / constants (scalar HWDGE ring) ----
            w1s = wp.tile([80, 5 * 96], F32R, tag="w1")
            nc.scalar.dma_start(w1s[:], w1_t[:])
            wls = {}
            for l in (2, 3, 4, 5, 6):
                wls[l] = wp.tile([96, 5 * 96], F32R, tag=f"w{l}", name=f"w{l}sb")
                nc.scalar.dma_start(wls[l][:], wl_t[l][:])
            w7s = wp.tile([96, 3 * 128], F32R, tag="w7")
            nc.scalar.dma_start(w7s[:], w7_t[:])
            lw1s = wp.tile([128, 128], F32R, tag="lw1")
            nc.scalar.dma_start(lw1s[:], lw1_t[:])
            lw2s = wp.tile([128, 64], F32R, tag="lw2")
            nc.scalar.dma_start(lw2s[:], lw2_t[:])
            lw3s = wp.tile([64, 5], F32R, tag="lw3")
            nc.scalar.dma_start(lw3s[:], lw3_t[:])
            bs = wp.tile([128, 10], F32, tag="bias")
            nc.scalar.dma_start(bs[:], bias_t[:])
            fms = wp.tile([128, N_SLOTS * T7u], F32, tag="fmask")
            nc.scalar.dma_start(fms[:], fmask_t[:])

            # ---- PE warm-up: keep the HAM activity window busy while the
            # first x tiles stream in (outputs are never read) ----
            wtile = ap_.tile([80, 512], F32R, tag="wtile")
            nc.gpsimd.memset(wtile[:].bitcast(F32), 0.0)
            ps_w = pp.tile([96, 480], F32, tag="conv")
            for wi in range(14):
                nc.tensor.matmul(
                    ps_w[0:96, 0:480],
                    wtile[0:80, 0:96],
                    wtile[0:80, 0:480],
                    start=True,
                    stop=True,
                )

            # ---- concatenated slot-interleaved tail buffers (phase-split) ----
            p4Ecat = cp.tile([96, N_SLOTS * P4E], F32R, tag="p4Ecat")
            p4Ocat = cp.tile([96, N_SLOTS * P4O], F32R, tag="p4Ocat")
            nc.gpsimd.memset(p4Ecat[:].bitcast(F32), 0.0)
            nc.gpsimd.memset(p4Ocat[:].bitcast(F32), 0.0)
            p5Ecat = cp.tile([96, N_SLOTS * P5E], F32R, tag="p5Ecat")
            p5Ocat = cp.tile([96, N_SLOTS * P5O], F32R, tag="p5Ocat")
            b6Ecat = cp.tile([96, N_SLOTS * P6E], F32R, tag="b6Ecat")
            b6Ocat = cp.tile([96, N_SLOTS * P6O], F32R, tag="b6Ocat")
            b7cat = cp.tile([128, N_SLOTS * T7u], F32, tag="b7cat")

            def act(dst_ap, src_ap, bias_col, func=AF.Relu, prange=96):
                nc.scalar.activation(
                    dst_ap, src_ap, func, bias=bs[0:prange, bias_col : bias_col + 1]
                )

            # per-slot phase-split activation buffers
            bufEt = {}
            bufOt = {}

            def phase_evict(bEO, half, ts_, cols, ps, bias_col):
                # one ACT: in [96, 2, cols/2] (phase, t) <- psum pairs;
                # out col = phase*half + ts_/2 + t
                h = cols // 2
                inv = ps[0:96, 0:cols].rearrange("p (t q) -> p q t", q=2)
                outv = bEO[0:96, 0 : 2 * half].rearrange(
                    "p (q t) -> p q t", q=2)[:, :, ts_ // 2 : ts_ // 2 + h]
                act(outv, inv, bias_col)

            def emit_l1(j):
                x2 = x2t[j]
                t1 = T1[j]
                bEO = ap_.tile([96, t1], F32R, tag="b1", name=f"b1_{j}")
                half = t1 // 2
                for ts_ in range(0, t1, 2048):
                    cols = min(2048, t1 - ts_)
                    ps = pp.tile([96, 2048], F32, tag="conv", name=f"ps1_{j}_{ts_}")
                    for g in range(5):
                        lhsT = w1s[:, 96 * g : 96 * (g + 1)]
                        for cs in range(0, cols, 512):
                            cn = min(512, cols - cs)
                            t_off = ts_ + cs
                            nc.tensor.matmul(
                                ps[0:96, cs : cs + cn],
                                lhsT,
                                x2[0:80, t_off + g : t_off + g + cn],
                                start=(g == 0),
                                stop=(g == 4),
                            )
                    phase_evict(bEO, half, ts_, cols, ps, 0)
                bufEt[(1, j)] = bEO

            # tap order for K=5 on phase-split input: (E,0),(O,0),(E,1),(O,1),(E,2)
            PHASES5 = ((0, 0), (1, 0), (0, 1), (1, 1), (0, 2))

            def emit_conv(j, lsrc, ldst, w_sb, tin_all, tout_all, bias_col):
                tout = tout_all[j]
                srcEO = bufEt[(lsrc, j)]
                shalf = tin_all[j] // 2
                bEO = ap_.tile([96, tout], F32R, tag=f"b{ldst}", name=f"b{ldst}_{j}")
                half = tout // 2
                for ts_ in range(0, tout, 2048):
                    cols = min(2048, tout - ts_)
                    ps = pp.tile([96, 2048], F32, tag="conv", name=f"psc{ldst}_{j}_{ts_}")
                    for g, (ph, d) in enumerate(PHASES5):
                        lhsT = w_sb[:, 96 * g : 96 * (g + 1)]
                        off = ph * shalf + d
                        for cs in range(0, cols, 512):
                            cn = min(512, cols - cs)
                            t_off = ts_ + cs
                            nc.tensor.matmul(
                                ps[0:96, cs : cs + cn],
                                lhsT,
                                srcEO[0:96, off + t_off : off + t_off + cn],
                                start=(g == 0),
                                stop=(g == 4),
                            )
                    phase_evict(bEO, half, ts_, cols, ps, bias_col)
                bufEt[(ldst, j)] = bEO

            def emit_l4pool(j):
                t4 = T4[j]
                t4p = T4p[j]
                srcEO = bufEt[(3, j)]
                shalf = T3[j] // 2
                ps = pp.tile([96, 512], F32, tag="conv", name=f"ps4_{j}")
                for g, (ph, d) in enumerate(PHASES5):
                    off = ph * shalf + d
                    nc.tensor.matmul(
                        ps[0:96, 0:t4],
                        wls[4][:, 96 * g : 96 * (g + 1)],
                        srcEO[0:96, off : off + t4],
                        start=(g == 0),
                        stop=(g == 4),
                    )
                # pool pairs are exactly (even, odd) psum columns
                tE = ap_.tile([96, t4p], F32, tag="t4e", name=f"t4e_{j}")
                tO = ap_.tile([96, t4p], F32, tag="t4o", name=f"t4o_{j}")
                nc.vector.tensor_copy(tE[:, 0:t4p], ps[0:96, 0 : 2 * t4p - 1 : 2])
                nc.vector.tensor_copy(tO[:, 0:t4p], ps[0:96, 1 : 2 * t4p : 2])
                t4s = ap_.tile([96, t4p], F32, tag="t4s", name=f"t4s_{j}")
                nc.vector.tensor_add(t4s[:, 0:t4p], tE[0:96, 0:t4p], tO[0:96, 0:t4p])
                # relu(e+o+2*b4), phase-split into slot-interleaved cat buffers
                nE = (t4p + 1) // 2
                nO = t4p // 2
                act(p4Ecat[0:96, j : 8 * (nE - 1) + j + 1 : 8],
                    t4s[0:96, 0 : 2 * nE - 1 : 2], 3)
                if nO:
                    act(p4Ocat[0:96, j : 8 * (nO - 1) + j + 1 : 8],
                        t4s[0:96, 1 : 2 * nO : 2], 3)

            # ---- batched tail layers ----
            ps5h = {}

            def emit_tail_l5(s0):
                p4Ev = p4Ecat[0:96, :].rearrange("p (t s) -> p t s", s=N_SLOTS)
                p4Ov = p4Ocat[0:96, :].rearrange("p (t s) -> p t s", s=N_SLOTS)
                ps5 = pp.tile([96, 4 * T5u], F32, tag="conv", name=f"ps5_{s0}")
                for g, (ph, d) in enumerate(PHASES5):
                    src = p4Ov if ph else p4Ev
                    nc.tensor.matmul(
                        ps5[0:96, :],
                        wls[5][:, 96 * g : 96 * (g + 1)],
                        src[:, d : d + T5u, s0 : s0 + 4],
                        start=(g == 0),
                        stop=(g == 4),
                    )
                ps5h[s0] = ps5

            def emit_tail_pool5(s0):
                ps5 = ps5h[s0]
                buf5 = ap_.tile([96, 4 * T5u], F32, tag="b5", name=f"b5_{s0}")
                nc.vector.tensor_copy(buf5[:], ps5[0:96, :])
                b5v = buf5[:].rearrange("p (t s) -> p t s", s=4)
                tmp5 = ap_.tile([96, 4 * T5pu], F32, tag="t5", name=f"t5_{s0}")
                nc.vector.tensor_add(
                    tmp5[:].rearrange("p (t s) -> p t s", s=4),
                    b5v[:, 0 : 2 * T5pu : 2, :],
                    b5v[:, 1 : 2 * T5pu : 2, :],
                )
                t5v = tmp5[:].rearrange("p (u s) -> p u s", s=4)
                act(p5Ecat[0:96, :].rearrange("p (u s) -> p u s", s=N_SLOTS)[:, 0:P5E, s0 : s0 + 4],
                    t5v[:, 0 : 2 * P5E - 1 : 2, :], 4)
                act(p5Ocat[0:96, :].rearrange("p (u s) -> p u s", s=N_SLOTS)[:, 0:P5O, s0 : s0 + 4],
                    t5v[:, 1 : 2 * P5O : 2, :], 4)

            def emit_tail_rest():
                p5Ev = p5Ecat[0:96, :]
                p5Ov = p5Ocat[0:96, :]
                ps6 = pp.tile([96, N_SLOTS * T6u], F32, tag="conv")
                for g, (ph, d) in enumerate(PHASES5):
                    src = p5Ov if ph else p5Ev
                    nc.tensor.matmul(
                        ps6[0:96, :],
                        wls[6][:, 96 * g : 96 * (g + 1)],
                        src[:, 8 * d : 8 * (d + T6u)],
                        start=(g == 0),
                        stop=(g == 4),
                    )
                ps6v = ps6[0:96, 0 : 8 * T6u].rearrange("p (t s) -> p t s", s=N_SLOTS)
                act(b6Ecat[0:96, :].rearrange("p (v s) -> p v s", s=N_SLOTS),
                    ps6v[:, 0 : 2 * P6E - 1 : 2, :], 5)
                act(b6Ocat[0:96, :].rearrange("p (v s) -> p v s", s=N_SLOTS),
                    ps6v[:, 1 : 2 * P6O : 2, :], 5)

                ps7 = pp.tile([128, N_SLOTS * T7u], F32, tag="conv")
                for g, (src, d) in enumerate(((b6Ecat, 0), (b6Ocat, 0), (b6Ecat, 1))):
                    nc.tensor.matmul(
                        ps7[0:128, :],
                        w7s[:, 128 * g : 128 * (g + 1)],
                        src[0:96, 8 * d : 8 * (d + T7u)],
                        start=(g == 0),
                        stop=(g == 2),
                    )
                act(b7cat[:], ps7[0:128, :], 6, prange=128)

            # ---- paired slot emission, smallest capacities first ----
            pair_order = [(7, 6), (5, 4), (3, 2), (1, 0)]
            for pi, (a, b) in enumerate(pair_order):
                emit_l1(a)
                emit_l1(b)
                if pi + 1 < len(pair_order):
                    emit_load(pair_order[pi + 1][0])
                    emit_load(pair_order[pi + 1][1])
                emit_conv(a, 1, 2, wls[2], T1, T2, 1)
                emit_conv(b, 1, 2, wls[2], T1, T2, 1)
                emit_conv(a, 2, 3, wls[3], T2, T3, 2)
                emit_conv(b, 2, 3, wls[3], T2, T3, 2)
                emit_l4pool(a)
                emit_l4pool(b)

            emit_tail_l5(4)
            emit_tail_l5(0)
            emit_tail_pool5(4)
            emit_tail_pool5(0)
            emit_tail_rest()

            # ---- ragged masked max + MLP head ----
            tmpm = ap_.tile([128, N_SLOTS * T7u], F32, tag="tm")
            nc.vector.tensor_add(tmpm[:], b7cat[:], fms[:])
            xmax = ap_.tile([128, N_SLOTS], F32R, tag="xmax")
            nc.vector.reduce_max(
                xmax[:],
                tmpm[:].rearrange("p (t s) -> p s t", s=N_SLOTS),
                axis=AX.X,
            )

            psm1 = pp.tile([128, N_SLOTS], F32, tag="conv")
            nc.tensor.matmul(psm1[0:128, :], lw1s[:], xmax[:], start=True, stop=True)
            h1 = ap_.tile([128, N_SLOTS], F32R, tag="h1")
            act(h1[:], psm1[0:128, :], 7, prange=128)

            psm2 = pp.tile([64, N_SLOTS], F32, tag="conv")
            nc.tensor.matmul(psm2[0:64, :], lw2s[:], h1[:], start=True, stop=True)
            h2 = ap_.tile([64, N_SLOTS], F32R, tag="h2")
            act(h2[:], psm2[0:64, :], 8, prange=64)

            psm3 = pp.tile([5, N_SLOTS], F32, tag="conv")
            nc.tensor.matmul(psm3[0:5, :], lw3s[:], h2[0:64, :], start=True, stop=True)
            outsb = ap_.tile([5, N_SLOTS], F32, tag="osb")
            nc.vector.tensor_scalar_add(outsb[:], psm3[0:5, :], bs[0:5, 9:10])
            nc.sync.dma_start(out_t[:], outsb[:])

    nc.compile()
    return nc


def _prep_x(x, b, cap):
    """Host-side input re-layout: phase-major polyphase [80, cap//2]."""
    xb = np.asarray(x[b, :, :cap], np.float32)
    th = cap // 2
    return np.concatenate([xb[:, 0 : 2 * th : 2], xb[:, 1 : 2 * th : 2]], axis=0)


def _prep_weights(inp):
    """Host-side weight/bias re-layout (all tiny)."""
    w = {}
    w1 = np.asarray(inp["w1"], np.float32)  # [96, 40, 10]
    # polyphase phase-major rows (p*40+c), cols (m*96+o): W1[o, c, 2m+p]
    w["w1s"] = np.ascontiguousarray(
        w1.transpose(1, 2, 0).reshape(40, 5, 2, 96).transpose(2, 0, 1, 3).reshape(80, 480)
    )
    for l, scale in ((2, 1.0), (3, 1.0), (4, 1.0), (5, 0.5), (6, 0.5)):
        wl = np.asarray(inp[f"w{l}"], np.float32)  # [96, 96, 5]
        w[f"w{l}s"] = np.ascontiguousarray(wl.transpose(1, 2, 0).reshape(96, 480) * scale)
    w7 = np.asarray(inp["w7"], np.float32)  # [128, 96, 3]
    w["w7s"] = np.ascontiguousarray(w7.transpose(1, 2, 0).reshape(96, 384))
    w["lw1T"] = np.ascontiguousarray(np.asarray(inp["lw1"], np.float32).T)  # [128,128]
    w["lw2T"] = np.ascontiguousarray(np.asarray(inp["lw2"], np.float32).T)  # [128,64]
    w["lw3T"] = np.ascontiguousarray(np.asarray(inp["lw3"], np.float32).T)  # [64,5]

    biases = np.zeros((128, 10), np.float32)
    biases[0:96, 0] = np.asarray(inp["b1"], np.float32)
    biases[0:96, 1] = np.asarray(inp["b2"], np.float32)
    biases[0:96, 2] = np.asarray(inp["b3"], np.float32)
    biases[0:96, 3] = 2.0 * np.asarray(inp["b4"], np.float32)
    biases[0:96, 4] = 2.0 * np.asarray(inp["b5"], np.float32)
    biases[0:96, 5] = np.asarray(inp["b6"], np.float32)
    biases[0:128, 6] = np.asarray(inp["b7"], np.float32)
    biases[0:128, 7] = np.asarray(inp["lb1"], np.float32)
    biases[0:64, 8] = np.asarray(inp["lb2"], np.float32)
    biases[0:5, 9] = np.asarray(inp["lb3"], np.float32)
    w["biases"] = biases
    return w


def _schedule(len_mask):
    """Sort samples by length desc, deal round-robin: core c, slot j gets
    sample order[8j + c].  Slot capacity = rank-group max."""
    lens = np.asarray(len_mask, np.int64).clip(1, T_FULL)
    order = np.argsort(-lens, kind="stable")
    sample_of = np.zeros((N_CORES, N_SLOTS), np.int64)
    caps = []
    for j in range(N_SLOTS):
        grp = order[j * N_CORES : (j + 1) * N_CORES]
        for c in range(N_CORES):
            sample_of[c, j] = grp[c]
        cap = int(lens[grp].max())
        cap = max(cap, 1312)  # keep the whole chain >= 1 frame
        # round up to a multiple of 32 so T1..T4 are all even
        # (fp32r matmuls require an even moving-operand size)
        cap = min(((cap + 31) // 32) * 32, T_FULL)
        caps.append(cap)
    return order, sample_of, caps


def _make_in_maps(inputs, sample_of, caps):
    x = np.asarray(inputs["x_input"], np.float32)
    len_mask = np.asarray(inputs["len_mask"], np.int32)
    _, _, _, _, T7u = _uniform_tail(caps)
    w = _prep_weights(inputs)
    in_maps = []
    for c in range(N_CORES):
        m = dict(w)
        # slot-interleaved mask layout: column = t*8 + s
        fm2 = np.full((T7u, N_SLOTS), NEG, np.float32)
        for j in range(N_SLOTS):
            bidx = int(sample_of[c, j])
            m[f"x{j}"] = _prep_x(x, bidx, caps[j])
            lv7 = _chain(int(max(min(len_mask[bidx], T_FULL), 1312)))[8]
            lv7 = max(min(lv7, T7u), 1)
            fm2[0:lv7, j] = 0.0
        fmask = fm2.reshape(-1)
        m["fmask"] = np.ascontiguousarray(
            np.broadcast_to(fmask[None, :], (128, N_SLOTS * T7u))
        )
        in_maps.append(m)
    return in_maps


def _ensure_ntff_hook():
    """The agent image lacks ``antenv.axon_hooks``; seed a shim so
    ``run_bass_kernel_spmd(trace=True)`` can reach the axon NTFF profiler."""
    import types

    if "antenv.axon_hooks" in sys.modules:
        return
    try:
        from trn_agent_boot.trn_boot import _ntff_profile_via_ctypes

        hook = _ntff_profile_via_ctypes("/opt/axon/libaxon_pjrt.so")
    except Exception:
        hook = None
    mod = types.ModuleType("antenv.axon_hooks")
    state = {"hook": hook}
    mod.get_axon_ntff_profile_hook = lambda: state["hook"]
    mod.set_axon_ntff_profile_hook = lambda h: state.update(hook=h)
    sys.modules["antenv.axon_hooks"] = mod


_LDW_PATCHED = False


def _enable_ldw_opt():
    """Turn on walrus's LDWEIGHTS dedup (drops redundant weight reloads for
    back-to-back same-weight matmuls).  Verified bit-identical results."""
    global _LDW_PATCHED
    if _LDW_PATCHED:
        return
    try:
        import concourse.bass_utils as bu

        _orig = bu.run_command

        def run_command_ldw(argv, **kw):
            argv = [
                "--enable-ldw-opt=true" if a == "--enable-ldw-opt=false" else a
                for a in argv
            ]
            return _orig(argv, **kw)

        bu.run_command = run_command_ldw
        _LDW_PATCHED = True
    except Exception:
        pass


def _run(inputs, trace=False):
    if trace:
        _ensure_ntff_hook()
    _enable_ldw_opt()
    len_mask = np.asarray(inputs["len_mask"], np.int32)
    order, sample_of, caps = _schedule(len_mask)
    nc = _build_program(caps)
    in_maps = _make_in_maps(inputs, sample_of, caps)
    res = run_bass_kernel_spmd(
        nc, in_maps, core_ids=list(range(N_CORES)), trace=trace
    )
    out = np.zeros((B, 5), np.float32)
    for c in range(N_CORES):
        o = res.results[c]["out"]  # [5, 8]
        for j in range(N_SLOTS):
            out[int(sample_of[c, j])] = o[:, j]
    return out, res


def kernel(**inputs):
    out, _ = _run(inputs, trace=False)
    return out
